# revision 34
# baseline (speedup 1.0000x reference)
"""Capsule-routing kernel for Trainium2 (8 NeuronCores, SPMD data-parallel over batch).

Algorithm restructure (hat-free):
  reference materializes hat = x @ W  (B, N, K*M) = 512 MiB and routes over it.
  All routing contractions reassociate through x and W directly:
    iter 1: c uniform = 1/K  ->  y1 = (1/K) * sum_n x
    s_i  = (c_i^T x) . W_blocks ;  o_i = squash(s_i)
    u_i  = W_blocks . o_i ;  logits_{i+1} = x . (u_1+...+u_i)
  This cuts FLOPs ~8x and avoids 512 MiB of DRAM traffic entirely.

Implementation: raw Block-mode Bass (no Tile scheduler) with hand-placed
semaphores. Every cross-engine dependency is a standalone wait_ge, so
fp32r matmuls (single sync-wait slot in the S3_LW descriptor) never carry
multiple waits. One body per engine; per-phase sem values are computed
inline while the per-engine op lists are built in global program order.

Layouts per core (4 batch items):
  - logits/c: (128 part = (b,k) b-major, N free); softmax denominator via a
    block-diag-ones matmul (partition-group reduce+broadcast in one op)
  - c transposed chunk-wise on the PE to (n, (b,k)) for y-matmul stationaries
  - yT columns permuted to k-major so the s-matmul runs as 4 quadrant
    matmuls whose diagonal blocks land in one 512-wide psum bank
  - squash runs on the masked waste (no compaction); u is built by a
    block-diagonal zero-padded stationary (oblk) scattered via PE transposes
"""

import os
import numpy as np

NCORES = 8
B_FULL, N, D = 32, 2048, 256
K, M = 32, 64
KM = K * M
BPC = B_FULL // NCORES
EPS = 1e-7
NCHUNK = N // 128   # 16
DCHUNK = D // 128   # 2
USE_F32R = os.environ.get("KERNEL_F32R", "1") == "1"

LAST_EXEC_NS = None
_CACHED = {}


def _build_nc():
    import concourse.bass as bass
    from concourse import mybir

    nc = bass.Bass()
    f32 = mybir.dt.float32
    AF = mybir.ActivationFunctionType
    ALU = mybir.AluOpType

    xp = nc.declare_dram_parameter("xp", [BPC, N, D], f32, isOutput=False)
    xt = nc.declare_dram_parameter("xt", [BPC, D, N], f32, isOutput=False)
    wsb = nc.declare_dram_parameter("wsb", [D, KM], f32, isOutput=False)
    wt = nc.declare_dram_parameter("wt", [KM, D], f32, isOutput=False)
    ctc = nc.declare_dram_parameter("ctc", [128, K], f32, isOutput=False)
    e4 = nc.declare_dram_parameter("e4", [128, 128], f32, isOutput=False)
    id128 = nc.declare_dram_parameter("id128", [128, 128], f32, isOutput=False)
    masks = nc.declare_dram_parameter("masks", [128, 32], f32, isOutput=False)
    zs = nc.declare_dram_parameter("zs", [128, NCHUNK * 128], f32, isOutput=False)
    out = nc.declare_dram_parameter("out", [BPC, K, M], f32, isOutput=True)
    oscr = nc.dram_tensor("oscr", [128, M], f32)

    def r32(ap):
        return ap.bitcast(mybir.dt.float32r) if USE_F32R else ap

    w32 = r32

    # ---- SBUF ----
    x_sb = [nc.alloc_sbuf_tensor(f"x{b}", [128, NCHUNK * D], f32).ap() for b in range(BPC)]
    xt_sb = [
        [nc.alloc_sbuf_tensor(f"xt{b}_{dc}", [128, N], f32).ap() for dc in range(DCHUNK)]
        for b in range(BPC)
    ]
    w_sb = [nc.alloc_sbuf_tensor(f"w{dc}", [128, KM], f32).ap() for dc in range(DCHUNK)]
    wt_sb = nc.alloc_sbuf_tensor("wtt", [128, NCHUNK * D], f32).ap()
    ctc_sb = nc.alloc_sbuf_tensor("ctcs", [128, K], f32).ap()
    e4_sb = nc.alloc_sbuf_tensor("e4s", [128, 128], f32).ap()
    id_sb = nc.alloc_sbuf_tensor("idm", [128, 128], f32).ap()
    msk_sb = nc.alloc_sbuf_tensor("msk", [128, 32], f32).ap()
    oblk = nc.alloc_sbuf_tensor("oblk", [128, NCHUNK * 128], f32).ap()
    ublk = nc.alloc_sbuf_tensor("ublk", [128, 8 * 128], f32).ap()
    expb = nc.alloc_sbuf_tensor("expb", [128, N], f32).ap()
    rden = nc.alloc_sbuf_tensor("rdenc", [128, N], f32).ap()
    cT = nc.alloc_sbuf_tensor("cT", [128, NCHUNK * 128], f32).ap()
    # lifetime-disjoint aliases (SBUF pressure): s-tail scratch reuses expb/cT
    sraw = expb
    sqs = cT
    owst = cT
    y_sb = nc.alloc_sbuf_tensor("ysb", [128, BPC * D], f32).ap()
    yT = nc.alloc_sbuf_tensor("yT", [128, D], f32).ap()
    ssq = nc.alloc_sbuf_tensor("ssq", [128, K], f32).ap()
    
    st = nc.alloc_sbuf_tensor("stt", [128, 8], f32).ap()
    u_sb = nc.alloc_sbuf_tensor("usb", [128, D], f32).ap()
    o_cmp = nc.alloc_sbuf_tensor("ocmp", [128, M], f32).ap()

    # ---- PSUM (8 banks total) ----
    big_ps = nc.alloc_psum_tensor("bigp", [128, 2048], f32).ap()   # 4 banks
    t_ps = [nc.alloc_psum_tensor(f"tp{i}", [128, 128], f32).ap() for i in range(2)]  # 2
    u_ps = nc.alloc_psum_tensor("up", [128, 256], f32).ap()        # 1

    # ---- program construction ----
    ops = {"SP": [], "PE": [], "ACT": [], "DVE": []}
    cnt = {"A": 0, "B": 0, "O": 0, "P": 0, "V": 0, "C": 0}
    waited = {e: {} for e in ops}

    def emit(eng, fn, waits=(), inc=None, inc_by=1):
        waits = list(waits)
        if eng == "DVE":
            # DVE pipeline: dependent back-to-back DVE ops need retirement
            # ordering; self-wait on the previous op's sem value.
            waits.append(("V", cnt["V"]))
        real = []
        for sem_key, val in waits:
            if val > 0 and waited[eng].get(sem_key, -1) < val:
                real.append((sem_key, val))
                waited[eng][sem_key] = val
        ops[eng].append((tuple(real), fn, inc, inc_by))
        if inc is not None:
            cnt[inc] += inc_by

    def pe_mm(outap, lhsT, rhs, start, stop, tp=None, waits=()):
        emit("PE",
             lambda o=outap, l=lhsT, r=rhs, s=start, t=stop,
             p=(tp if tp is not None and tp[1] == 96 else None):
             nc.tensor.matmul(o, r32(l), r32(r), start=s, stop=t, tile_position=p),
             waits=waits, inc="P")

    def pe_tr(outap, inap, waits=()):
        p = inap.partition_size()
        emit("PE",
             lambda o=outap, i=inap, p=p: nc.tensor.transpose(o, i, id_sb[0:p, 0:p]),
             waits=waits, inc="P")

    # ---- loads ----
    def dma(outap, inap, grp, waits=()):
        emit("SP", lambda o=outap, i=inap: nc.sync.dma_start(out=o, in_=i),
             waits=waits, inc=grp, inc_by=16)

    dma(w32(ctc_sb), w32(ctc[:]), "A")
    dma(w32(id_sb), w32(id128[:]), "A")
    dma(msk_sb, masks[:], "A")
    for dc in range(DCHUNK):
        dma(w32(w_sb[dc]), w32(wsb[dc * 128:(dc + 1) * 128, :]), "A")
    dma(w32(wt_sb.rearrange("p (t d) -> p t d", d=D)),
        w32(wt.rearrange("(t p) d -> p t d", p=128)), "A")
    dma(w32(oblk), w32(zs[:]), "A")
    dma(w32(ublk), w32(zs[:, 0:8 * 128]), "A")
    A_X = []
    for b in range(BPC):
        dma(w32(x_sb[b].rearrange("p (j d) -> p j d", d=D)),
            w32(xp[b].rearrange("(j p) d -> p j d", p=128)), "A")
        A_X.append(cnt["A"])
    A_FULL = cnt["A"]
    dma(w32(e4_sb), w32(e4[:]), "B")
    for b in range(BPC):
        for dc in range(DCHUNK):
            dma(w32(xt_sb[b][dc]), w32(xt[b, dc * 128:(dc + 1) * 128, :]), "B")
    B_FULL_V = cnt["B"]


    # ---- iterations ----
    for it in range(3):
        if it == 0:
            for b in range(BPC):
                for j in range(NCHUNK):
                    pe_mm(big_ps[0:K, 1024 + b * D: 1024 + (b + 1) * D], ctc_sb,
                          x_sb[b][:, j * D:(j + 1) * D],
                          start=(j == 0), stop=(j == NCHUNK - 1),
                          waits=[("A", A_X[b])])
        else:
            # logits: contraction over (b', d) with block-diag zero-padded ublk
            wv = cnt["V"]
            for sl in range(4):
                for ch in range(8):
                    pe_mm(big_ps[:, sl * 512:(sl + 1) * 512],
                          ublk[:, ch * 128:(ch + 1) * 128],
                          xt_sb[ch // 2][ch % 2][:, sl * 512:(sl + 1) * 512],
                          start=(ch == 0), stop=(ch == 7),
                          waits=[("B", B_FULL_V), ("V", wv)])
            # softmax
            emit("ACT", lambda: nc.scalar.activation(w32(expb), big_ps, AF.Exp),
                 waits=[("P", cnt["P"])], inc="C")
            wc = cnt["C"]
            for sl in range(4):
                pe_mm(big_ps[:, sl * 512:(sl + 1) * 512], e4_sb,
                      expb[:, sl * 512:(sl + 1) * 512],
                      start=True, stop=True, waits=[("C", wc)])
            emit("DVE", lambda: nc.vector.reciprocal(rden, big_ps),
                 waits=[("P", cnt["P"])], inc="V")
            emit("DVE", lambda: nc.vector.tensor_mul(rden, expb, rden), inc="V")
            c_sb = rden
            # cT transposes (ping-pong psum slots) then y matmuls
            copy_v = {}
            for j in range(NCHUNK):
                w = [("V", copy_v[j - 2])] if j >= 2 else [("V", cnt["V"])]
                pe_tr(t_ps[j % 2], c_sb[:, j * 128:(j + 1) * 128], waits=w)
                trp = cnt["P"]
                emit("DVE",
                     lambda j=j: nc.vector.tensor_copy(
                         w32(cT[:, j * 128:(j + 1) * 128]), t_ps[j % 2]),
                     waits=[("P", trp)], inc="V")
                copy_v[j] = cnt["V"]
            wv = cnt["V"]
            for b in range(BPC):
                for j in range(NCHUNK):
                    pe_mm(big_ps[0:K, 1024 + b * D: 1024 + (b + 1) * D],
                          cT[:, j * 128 + b * K: j * 128 + (b + 1) * K],
                          x_sb[b][:, j * D:(j + 1) * D],
                          start=(j == 0), stop=(j == NCHUNK - 1),
                          waits=[("V", wv)])

        # ---- shared tail ----
        emit("DVE", lambda: nc.vector.tensor_copy(y_sb[0:K, :], big_ps[0:K, 1024:2048]),
             waits=[("P", cnt["P"])], inc="V")
        ycp_v = {}
        for b in range(BPC):
            for dc in range(DCHUNK):
                i = b * DCHUNK + dc
                w = [("V", ycp_v[i - 2])] if i >= 2 else [("V", cnt["V"])]
                pe_tr(t_ps[i % 2][:, 0:K],
                      y_sb[0:K, b * D + dc * 128: b * D + (dc + 1) * 128],
                      waits=w)
                trp = cnt["P"]
                emit("DVE",
                     lambda b=b, dc=dc, i=i: nc.vector.tensor_copy(
                         w32(yT[:, dc * 128:(dc + 1) * 128]
                             .rearrange("d (k g) -> d k g", g=BPC)[:, :, b]),
                         t_ps[i % 2][:, 0:K]),
                     waits=[("P", trp)], inc="V")
                ycp_v[i] = cnt["V"]
        wv = cnt["V"]
        for q in range(4):
            for dc in range(DCHUNK):
                pe_mm(big_ps[:, 512 * q:512 * (q + 1)],
                      yT[:, dc * 128:(dc + 1) * 128],
                      w_sb[dc][:, 512 * q:512 * (q + 1)],
                      start=(dc == 0), stop=(dc == DCHUNK - 1),
                      waits=[("V", wv)])
        emit("ACT", lambda: nc.scalar.activation(w32(sraw), big_ps, AF.Copy),
             waits=[("P", cnt["P"])], inc="C")
        emit("DVE", lambda: nc.vector.tensor_mul(w32(sqs), sraw, sraw),
             waits=[("C", cnt["C"])], inc="V")
        emit("DVE",
             lambda: nc.vector.tensor_reduce(
                 ssq, sqs.rearrange("p (k m) -> p k m", m=M),
                 axis=mybir.AxisListType.X, op=ALU.add),
             inc="V")
        emit("DVE", lambda: nc.vector.tensor_mul(ssq, ssq, msk_sb), inc="V")
        emit("DVE",
             lambda: nc.vector.tensor_reduce(
                 st[:, 0:1], ssq, axis=mybir.AxisListType.X, op=ALU.add),
             inc="V")
        emit("DVE", lambda: nc.vector.tensor_scalar_add(st[:, 5:6], st[:, 0:1], EPS),
             inc="V")
        emit("ACT", lambda: nc.scalar.activation(st[:, 1:2], st[:, 5:6], AF.Sqrt),
             waits=[("V", cnt["V"])], inc="C")
        emit("DVE",
             lambda: nc.vector.tensor_scalar_add(st[:, 2:3], st[:, 0:1], 0.5 + EPS),
             waits=[("C", cnt["C"])], inc="V")
        emit("DVE", lambda: nc.vector.reciprocal(st[:, 3:4], st[:, 2:3]), inc="V")
        emit("DVE", lambda: nc.vector.tensor_mul(st[:, 4:5], st[:, 1:2], st[:, 3:4]),
             inc="V")
        emit("DVE",
             lambda: nc.vector.tensor_scalar(w32(owst), sraw, st[:, 4:5], None, ALU.mult),
             inc="V")
        owst_v = cnt["V"]

        if it == 2:
            for k in range(K):
                emit("SP",
                     lambda k=k: nc.sync.dma_start(
                         out=o_cmp[4 * k:4 * (k + 1), :],
                         in_=owst[4 * k:4 * (k + 1), 64 * k:64 * k + 64]),
                     waits=[("V", owst_v)], inc="O", inc_by=16)
            ofirst = cnt["O"]
            emit("SP",
                 lambda: nc.sync.dma_start(out=oscr[:], in_=o_cmp),
                 waits=[("O", ofirst)], inc="O", inc_by=16)
            osecond = cnt["O"]
            emit("SP",
                 lambda: nc.sync.dma_start(
                     out=out.rearrange("b k m -> k b m"),
                     in_=oscr.rearrange("(k b) m -> k b m", b=BPC)),
                 waits=[("O", osecond)], inc="O", inc_by=16)
            emit("SP", lambda: None, waits=[("O", cnt["O"])])
            continue

        # u-step
        scat_v = {}
        for t in range(NCHUNK):
            w = [("V", scat_v[t - 2])] if t >= 2 else [("V", owst_v)]
            pe_tr(t_ps[t % 2], owst[:, 128 * t:128 * (t + 1)], waits=w)
            trp = cnt["P"]
            ke, ko = 2 * t, 2 * t + 1
            emit("DVE",
                 lambda t=t, ke=ke: nc.vector.tensor_copy(
                     w32(oblk[0:64, 128 * t + 4 * ke: 128 * t + 4 * ke + BPC]),
                     t_ps[t % 2][0:64, 4 * ke: 4 * ke + BPC]),
                 waits=[("P", trp)], inc="V")
            emit("DVE",
                 lambda t=t, ko=ko: nc.vector.tensor_copy(
                     w32(oblk[64:128, 128 * t + 4 * ko: 128 * t + 4 * ko + BPC]),
                     t_ps[t % 2][64:128, 4 * ko: 4 * ko + BPC]),
                 inc="V")
            scat_v[t] = cnt["V"]
        wv = cnt["V"]
        for t in range(NCHUNK):
            pe_mm(u_ps, oblk[:, t * 128:(t + 1) * 128], wt_sb[:, t * D:(t + 1) * D],
                  start=(t == 0), stop=(t == NCHUNK - 1), waits=[("V", wv)])
        emit("DVE", lambda: nc.vector.tensor_copy(u_sb, u_ps),
             waits=[("P", cnt["P"])], inc="V")
        wv = cnt["V"]
        utr_p = []
        for dc in range(DCHUNK):
            pe_tr(t_ps[dc], u_sb[:, dc * 128:(dc + 1) * 128], waits=[("V", wv)])
            utr_p.append(cnt["P"])
        # scatter uT chunks into block-diag ublk (cols b-major within chunk)
        for dc in range(DCHUNK):
            for b in range(BPC):
                ch = 2 * b + dc
                uv = ublk[:, ch * 128 + b * K: ch * 128 + (b + 1) * K]
                tv = t_ps[dc].rearrange("d (k g) -> d g k", g=BPC)[:, b, :]
                if it == 0:
                    emit("DVE", lambda uv=uv, tv=tv: nc.vector.tensor_copy(w32(uv), tv),
                         waits=[("P", utr_p[dc])], inc="V")
                else:
                    emit("DVE", lambda uv=uv, tv=tv: nc.vector.tensor_add(w32(uv), uv, tv),
                         waits=[("P", utr_p[dc])], inc="V")

    # ---- emission: one body per engine ----
    with (
        nc.semaphore("sA") as sA,
        nc.semaphore("sB") as sB,
        nc.semaphore("sO") as sO,
        nc.semaphore("sP") as sP,
        nc.semaphore("sV") as sV,
        nc.semaphore("sC") as sC,
        nc.Block() as block,
    ):
        sem_handles = {"A": sA, "B": sB, "O": sO, "P": sP, "V": sV, "C": sC}

        def run_ops(eng_name):
            def body(e):
                for waits, fn, inc, inc_by in ops[eng_name]:
                    for sem_key, val in waits:
                        e.wait_ge(sem_handles[sem_key], val)
                    inst = fn()
                    if inc is not None and inst is not None:
                        inst.then_inc(sem_handles[inc], inc_by)
            return body

        block.sync(run_ops("SP"))
        block.tensor(run_ops("PE"))
        block.scalar(run_ops("ACT"))
        block.vector(run_ops("DVE"))
    return nc


def _get_nc():
    if "nc" not in _CACHED:
        _CACHED["nc"] = _build_nc()
    return _CACHED["nc"]


def kernel(x, W):
    global LAST_EXEC_NS
    from concourse.bass_utils import run_bass_kernel_spmd

    x = np.ascontiguousarray(x, dtype=np.float32)
    W = np.ascontiguousarray(W, dtype=np.float32)
    assert x.shape == (B_FULL, N, D) and W.shape == (D, KM)

    nc = _get_nc()

    ctc = np.full((128, K), 1.0 / K, dtype=np.float32)
    e4 = np.kron(np.eye(BPC, dtype=np.float32), np.ones((K, K), dtype=np.float32))
    id128 = np.eye(128, dtype=np.float32)
    wt = np.ascontiguousarray(W.T)
    masks = np.zeros((128, 32), dtype=np.float32)
    for p in range(128):
        masks[p, p // BPC] = 1.0
    zs = np.zeros((128, 2048), dtype=np.float32)

    in_maps = []
    for i in range(NCORES):
        xs = np.ascontiguousarray(x[i * BPC:(i + 1) * BPC])
        xts = np.ascontiguousarray(xs.transpose(0, 2, 1))
        in_maps.append(
            {"xp": xs, "xt": xts, "wsb": W, "wt": wt, "ctc": ctc, "e4": e4,
             "id128": id128, "masks": masks, "zs": zs}
        )

    trace = os.environ.get("KERNEL_TRACE", "0") == "1"
    res = run_bass_kernel_spmd(nc, in_maps, list(range(NCORES)), trace=trace)
    LAST_EXEC_NS = res.exec_time_ns
    outs = [res.results[i]["out"] for i in range(NCORES)]
    return np.concatenate(outs, axis=0)
